# revision 26
# baseline (speedup 1.0000x reference)
"""Causal self-attention (single head) on 8 Trainium2 NeuronCores.

Sharding: 8 cores = 4 batches x 2 query-tile parity sets. Core c handles
batch (c % 4). Cores 0-3 take query tiles t in {15,13,...,1} (128 rows
each), cores 4-7 take t in {14,12,...,0}. Attention iteration i uses a
fixed causal extent E(i) = 16-2i k-tiles, so a single SPMD program
serves all cores; even-parity cores waste one fully-masked k-tile per
iteration.

Math: scores = x Wq^T Wk x^T = (x A) x^T with A = Wq^T Wk folded on the
host. The kernel computes Q'^T = A^T-blocks x x_q^T (64 units over the
core's own 1024 q rows) instead of the key-side G = A x^T (128 units
over all 2048 keys) -- halving the projection phase.

Pair-swap trick: for odd-parity cores the host swaps the two 128-col
tiles inside each 256-col half-quarter of x^T (and permutes the xn
k-tiles to match), so BOTH parities' own q-tiles sit at column offsets
{0, 256} of every 512-col quarter. P1 then reads its moving operand
directly out of the resident key matrix (xk_all) through a strided AP
and the separate 2MB x_q staging load disappears; each 256-col half
still holds an adjacent tile pair, so every causal extent stays
covered, the mask still lands on the last 256 columns of each extent
(odd cores: [tri|0], even cores: [tri|-inf]), and the q-tile just
lands pair-swapped in qT_sb (consumers index i^1).

All transposes (softmax weights W and U = W x) go through the DMA xbar
instead of the PE array. Loads are consolidated into ~17 large
contiguous dma_starts striped across both HWDGE rings: sequencers pay
~0.7us issue per dma_start, completion semaphores lag data by several
us (DMA engine 15 runs ~15% slow and is the long pole), and the first
xbar transpose's guard waits on every previously-emitted DMA, gating
U(0). P1 accumulates dt-outer so each PSUM bank's copy overlaps the
next bank's matmuls. Iterations are software-pipelined: step j emits
scores(j), U(j-1), Z(j-2). Softmax skips max-subtraction (scores/32
stay in a safe exp range); row sums come free via activation
accum_out. Z emits its column halves sequentially so scale+store of
the first overlaps matmuls of the second; the final step splits
quarters to shrink the exposed tail further.
"""

import sys

for _p in ("/opt/trn_rl_repo", "/root/.axon_site/_ro/trn_rl_repo"):
    if _p not in sys.path:
        sys.path.append(_p)

import numpy as np

import concourse.bass as bass  # noqa: F401
import concourse.mybir as mybir
import concourse.tile as tile
from concourse import bacc
from concourse.bass_utils import run_bass_kernel_spmd

F32 = mybir.dt.float32
F16 = mybir.dt.float16

BATCH, SEQ, D, P = 4, 2048, 1024, 1024
N_CORES = 8
QT = 128          # query tile rows
KTL = 128         # key tile
NBLK = 512        # matmul moving free dim
ND = D // 128     # 8 d-tiles
NKT = SEQ // KTL  # 16 k-tiles
NQT = 8           # q-tiles per core
SCALE = 1.0 / float(np.sqrt(P))
NEG = -1e9

# processing order over program q-tile index i (extent = 16-2i). The
# first two steps are the smallest extents so their exp/transpose
# chains clear early; scores stay ~4 steps ahead of U.
ORDER = (7, 6, 0, 1, 2, 3, 4, 5)


def _extent(i):
    return 16 - 2 * i


def _chunks(width):
    out = []
    w = width
    while w >= NBLK:
        out.append(NBLK)
        w -= NBLK
    if w:
        assert w == 256, w
        out.append(256)
    return out


def build_program():
    nc = bacc.Bacc("TRN2", target_bir_lowering=False)

    # All inputs are host-pre-tiled to match the SBUF destination layout
    # exactly, so every DMA load is contiguous on both sides (2-8KB runs
    # per partition, few descriptors). 1KB-run slicing costs ~10us of
    # HWDGE descriptor generation per 2MB and stalls the issuing ring.
    xkq = nc.dram_tensor("xkq", [4, 128, ND, 512], F16, kind="ExternalInput")
    xn2 = nc.dram_tensor("xn2", [128, NKT, D], F16, kind="ExternalInput")
    AT = nc.dram_tensor("AT", [D, D], F16, kind="ExternalInput")
    wv2 = nc.dram_tensor("wv2", [128, ND, P], F16, kind="ExternalInput")
    mask = nc.dram_tensor("mask", [QT, 256], F32, kind="ExternalInput")
    out = nc.dram_tensor("out", [NQT * QT, P], F32, kind="ExternalOutput")

    at_r = AT.rearrange("(et ep) d -> ep et d", ep=128)

    with tile.TileContext(nc) as tc:
        with (
            tc.tile_pool(name="resident", bufs=1) as resident,
            tc.tile_pool(name="wrow", bufs=3) as wrow,
            tc.tile_pool(name="tpool", bufs=3) as tpool,
            tc.tile_pool(name="small", bufs=6) as small,
            tc.tile_pool(name="outp", bufs=2) as outp,
        ):
            at_sb = resident.tile([128, ND, D], F16)      # A' = Wq^T Wk
            qT_sb = resident.tile([128, ND, NQT * QT], F16)   # Q'^T [d, q]
            xk_all = resident.tile([128, 4, ND, 512], F16)  # x.T, k-quarters
            xn_all = resident.tile([128, NKT, D], F16)    # x natural [k, d]
            wv_sb = resident.tile([128, ND, P], F16)
            mask_sb = resident.tile([QT, 256], F32)
            cbias = resident.tile([QT, 1], F32)
            nc.vector.memset(cbias, -4.0)

            # Consolidated loads, striped across both rings in consumer
            # order. P1 runs et-outer inside each xk quarter, consuming
            # (at[et], xk-quarter pieces) as they stream: q0 + all at
            # slices lead (P1's every quarter touches every at[et]),
            # then q1-q3 split in halves so their completion semaphores
            # land ahead of P1's ~8.5us-per-quarter pace; xn/wv follow
            # for U(7)/Z(7) at ~50us.
            nc.sync.dma_start(out=at_sb[:, 0, 0:256], in_=at_r[:, 0, 0:256])
            nc.scalar.dma_start(out=xk_all[:, 0, 1:2, :],
                                in_=xkq[0, :, 1:2, :])
            nc.sync.dma_start(out=xk_all[:, 0, 0:1, :], in_=xkq[0, :, 0:1, :])
            nc.scalar.dma_start(out=xk_all[:, 0, 2:4, :],
                                in_=xkq[0, :, 2:4, :])
            nc.sync.dma_start(out=at_sb[:, 0, 256:D], in_=at_r[:, 0, 256:D])
            nc.scalar.dma_start(out=at_sb[:, 1, :], in_=at_r[:, 1, :])
            nc.sync.dma_start(out=at_sb[:, 2, :], in_=at_r[:, 2, :])
            nc.scalar.dma_start(out=xk_all[:, 0, 4:ND, :],
                                in_=xkq[0, :, 4:ND, :])
            nc.sync.dma_start(out=at_sb[:, 4, :], in_=at_r[:, 4, :])
            nc.scalar.dma_start(out=at_sb[:, 3, :], in_=at_r[:, 3, :])
            nc.sync.dma_start(out=at_sb[:, 6, :], in_=at_r[:, 6, :])
            nc.scalar.dma_start(out=at_sb[:, 5, :], in_=at_r[:, 5, :])
            nc.scalar.dma_start(out=at_sb[:, 7, :], in_=at_r[:, 7, :])
            nc.sync.dma_start(out=xk_all[:, 1, 0:4, :], in_=xkq[1, :, 0:4, :])
            nc.sync.dma_start(out=xk_all[:, 1, 4:ND, :],
                              in_=xkq[1, :, 4:ND, :])
            nc.scalar.dma_start(out=mask_sb, in_=mask[:, :])
            nc.scalar.dma_start(out=xk_all[:, 2, 0:4, :],
                                in_=xkq[2, :, 0:4, :])
            nc.scalar.dma_start(out=xk_all[:, 2, 4:ND, :],
                                in_=xkq[2, :, 4:ND, :])
            nc.sync.dma_start(out=xk_all[:, 3, 0:4, :], in_=xkq[3, :, 0:4, :])
            nc.sync.dma_start(out=xk_all[:, 3, 4:ND, :],
                              in_=xkq[3, :, 4:ND, :])
            nc.scalar.dma_start(out=xn_all[:, 0:8, :], in_=xn2[:, 0:8, :])
            nc.sync.dma_start(out=xn_all[:, 8:NKT, :], in_=xn2[:, 8:NKT, :])
            nc.sync.dma_start(out=wv_sb[:, 0:4, :], in_=wv2[:, 0:4, :])
            nc.scalar.dma_start(out=wv_sb[:, 4:ND, :], in_=wv2[:, 4:ND, :])

            # --- Q'^T = A'-blocks x x_q, with x_q read straight out of
            # xk_all: quarter qq holds the core's own q-tiles at column
            # offsets 0 (program i=7-2qq) and 256 (i=6-2qq). et-outer
            # inside each quarter so the matmuls chase the streaming
            # at[et]/xk pieces; the 8 dt-banks complete together at the
            # quarter's end and their single [128,256] copies overlap
            # the next quarter's matmuls. The copy lands the pair
            # contiguously at qT columns (6-2qq)*128 (consumers index
            # i^1).
            # Quarters are processed in pairs (2j, 2j+1): for each (et,
            # dt) the stationary at-block loads ONCE and serves two
            # back-to-back N=256 matmuls (one per quarter), so
            # LDWEIGHTS fully hides and the pair streams at the
            # 256-cycle floor.
            with tc.tile_pool(name="p1ps", bufs=8, space="PSUM") as p1ps:
                for qq in range(4):
                    xq_ap = xk_all[:, qq, :, :].rearrange(
                        "ep et (h c) -> ep et h c", c=128)
                    banks = [p1ps.tile([128, 256], F32, tag="p1",
                                       name=f"p1b{qq}_{dt}")
                             for dt in range(ND)]
                    for et in range(ND):
                        for dt in range(ND):
                            nc.tensor.matmul(
                                banks[dt],
                                at_sb[:, et, dt * 128:(dt + 1) * 128],
                                xq_ap[:, et, 0:3:2, :],
                                start=(et == 0),
                                stop=(et == ND - 1),
                            )
                    c0 = (6 - 2 * qq) * QT
                    for dt in range(ND):
                        cp = (nc.scalar.copy if dt % 2 == 0
                              else nc.vector.tensor_copy)
                        cp(qT_sb[:, dt, c0:c0 + 256], banks[dt])

            # --- attention, software-pipelined: scores(j), U(j-1), Z(j-2)
            with (
                tc.tile_pool(name="sps", bufs=4, space="PSUM") as sps,
                tc.tile_pool(name="uzps", bufs=4, space="PSUM") as uzps,
            ):
                state = {}

                def emit_scores(i):
                    ext = _extent(i)
                    width = ext * KTL
                    iq = i ^ 1  # qT_sb stores each quarter's pair swapped
                    s_list = []
                    off = 0
                    for cw in _chunks(width):
                        s_ps = sps.tile([QT, NBLK], F32, tag="s")
                        psv = s_ps[:, :cw]
                        for dt in range(ND):
                            nc.tensor.matmul(
                                psv,
                                qT_sb[:, dt, iq * QT:(iq + 1) * QT],
                                xk_all[:, off // 512, dt, 0:cw],
                                start=(dt == 0),
                                stop=(dt == ND - 1),
                            )
                        s_list.append((psv, off, cw))
                        off += cw

                    # additive causal mask on the last 256 columns
                    last_ps, _, cw_l = s_list[-1]
                    nc.vector.tensor_add(
                        last_ps[:, cw_l - 256:cw_l],
                        last_ps[:, cw_l - 256:cw_l],
                        mask_sb,
                    )

                    # exp((s + m) * scale - 4) -> fp16 weights row; row
                    # sums free via accum_out
                    w_sb = wrow.tile([QT, SEQ], F16, tag="w")
                    lparts = small.tile([QT, 4], F32, tag="lp")
                    for ci, (psv, off_c, cw) in enumerate(s_list):
                        nc.scalar.activation(
                            w_sb[:, off_c:off_c + cw],
                            psv,
                            mybir.ActivationFunctionType.Exp,
                            scale=SCALE,
                            bias=cbias,
                            accum_out=lparts[:, ci:ci + 1],
                        )
                    lsum = small.tile([QT, 1], F32, tag="ls")
                    nc.vector.reduce_sum(
                        lsum, lparts[:, :len(s_list)],
                        axis=mybir.AxisListType.X)
                    rl = small.tile([QT, 1], F32, tag="rl")
                    nc.vector.reciprocal(rl, lsum)

                    # blocked transpose W -> W^T tiles via the DMA xbar
                    wT = tpool.tile([128, NKT, 128], F16, tag="wT")
                    nc.sync.dma_start_transpose(
                        out=wT[:, 0:ext, :], in_=w_sb[:, 0:width])
                    state[i] = dict(ext=ext, wT=wT, rl=rl)

                def emit_U(i):
                    st = state[i]
                    ext = st["ext"]
                    wT = st["wT"]
                    u0 = uzps.tile([QT, NBLK], F32, tag="uz")
                    u1 = uzps.tile([QT, NBLK], F32, tag="uz")
                    for kt in range(ext):
                        nc.tensor.matmul(
                            u0, wT[:, kt, :], xn_all[:, kt, 0:NBLK],
                            start=(kt == 0), stop=(kt == ext - 1),
                        )
                        nc.tensor.matmul(
                            u1, wT[:, kt, :], xn_all[:, kt, NBLK:D],
                            start=(kt == 0), stop=(kt == ext - 1),
                        )
                    u_sb = wrow.tile([QT, D], F16, tag="u")
                    nc.scalar.copy(u_sb[:, 0:NBLK], u0)
                    nc.vector.tensor_copy(u_sb[:, NBLK:D], u1)
                    uT = tpool.tile([128, ND, 128], F16, tag="uT")
                    nc.sync.dma_start_transpose(out=uT, in_=u_sb)
                    st["uT"] = uT

                def emit_Z(i, last=False):
                    st = state.pop(i)
                    uT = st["uT"]
                    rl = st["rl"]
                    o_sb = outp.tile([QT, P], F32, tag="o")
                    # column-chunked: each chunk's matmuls complete, then
                    # its scale+store overlap the next chunk's matmuls.
                    # The final step uses 4 chunks to shrink the exposed
                    # scale+store tail after the very last matmul.
                    csz = 256 if last else NBLK
                    for cc in range(P // csz):
                        zfull = uzps.tile([QT, NBLK], F32, tag="uz")
                        zc = zfull[:, 0:csz]
                        lo = cc * csz
                        for dt in range(ND):
                            nc.tensor.matmul(
                                zc, uT[:, dt, :],
                                wv_sb[:, dt, lo:lo + csz],
                                start=(dt == 0), stop=(dt == ND - 1),
                            )
                        nc.vector.tensor_scalar_mul(
                            o_sb[:, lo:lo + csz], zc, rl)
                        # scalar queue: keeps output stores from delaying
                        # the dma transposes on the sync queue
                        nc.scalar.dma_start(
                            out=out[i * QT:(i + 1) * QT, lo:lo + csz],
                            in_=o_sb[:, lo:lo + csz])

                for j in range(NQT):
                    emit_scores(ORDER[j])
                    if j >= 1:
                        emit_U(ORDER[j - 1])
                    if j >= 2:
                        emit_Z(ORDER[j - 2])
                emit_U(ORDER[NQT - 1])
                emit_Z(ORDER[NQT - 2])
                emit_Z(ORDER[NQT - 1], last=True)

    nc.compile()
    return nc


def _tiles_for_core(c):
    """Global 128-row query-tile indices, indexed by program i=0..7."""
    return [(15 - 2 * i) if c < 4 else (14 - 2 * i) for i in range(NQT)]


_PSWAP = (1, 0, 3, 2)


def _host_prep(inputs, Wq, Wk, Wv):
    x = np.asarray(inputs, dtype=np.float32)
    Wqf = np.asarray(Wq, dtype=np.float32)
    Wkf = np.asarray(Wk, dtype=np.float32)
    # scores = x A' x^T with A' = Wq^T Wk; the Q'^T matmul contracts over
    # A's rows (lhsT[e, d] = A'[e, d]), so A' itself is the stationary.
    ATm = np.ascontiguousarray((Wqf.T @ Wkf).astype(np.float16))
    WvT = np.ascontiguousarray(
        np.asarray(Wv, dtype=np.float32).T.astype(np.float16))

    qi = np.arange(QT)[:, None]
    ki = np.arange(128)[None, :]
    tri = np.where(qi >= ki, 0.0, NEG).astype(np.float32)
    # pair-swapped (odd-tile) cores: the 256-col tail of every extent is
    # [diag tile | older allowed tile]; natural (even-tile) cores:
    # [diag tile | future masked tile].
    mask_odd = np.concatenate([tri, np.zeros((QT, 128), np.float32)], axis=1)
    mask_evn = np.concatenate(
        [tri, np.full((QT, 128), NEG, np.float32)], axis=1)

    # pre-tile the bulk tensors to match the SBUF destination layouts so
    # the device DMA loads are contiguous on both sides
    wv2 = np.ascontiguousarray(
        WvT.reshape(ND, 128, P).transpose(1, 0, 2))  # [128, dt, p]

    in_maps = []
    cache = {}
    for c in range(N_CORES):
        b = c % 4
        swap = c < 4  # odd global tiles at offsets {128,384} -> pair-swap
        key = (b, swap)
        if key not in cache:
            xTb = x[b].T.astype(np.float16)              # [d, k]
            xq4 = xTb.reshape(ND, 128, 4, 4, 128)        # [et,ep,qq,h,128]
            if swap:
                xq4 = xq4[:, :, :, _PSWAP, :]
            xkq = np.ascontiguousarray(
                xq4.reshape(ND, 128, 4, 512).transpose(2, 1, 0, 3))
            xnat = x[b].astype(np.float16).reshape(NKT, 128, D)
            if swap:
                perm = [4 * (kt // 4) + _PSWAP[kt % 4] for kt in range(NKT)]
                xnat = xnat[perm]
            xn2 = np.ascontiguousarray(xnat.transpose(1, 0, 2))
            cache[key] = (xkq, xn2)
        xkq, xn2 = cache[key]
        in_maps.append({
            "xkq": xkq,
            "xn2": xn2,
            "AT": ATm,
            "wv2": wv2,
            "mask": mask_odd if swap else mask_evn,
        })
    return in_maps


def _host_gather(results):
    Z = np.empty((BATCH, SEQ, P), dtype=np.float32)
    for c in range(N_CORES):
        b = c % 4
        o = results[c]["out"]
        for i, t in enumerate(_tiles_for_core(c)):
            Z[b, t * QT:(t + 1) * QT, :] = o[i * QT:(i + 1) * QT, :]
    return Z


_NC_CACHE = None


def kernel(inputs, Wq, Wk, Wv):
    global _NC_CACHE
    if _NC_CACHE is None:
        _NC_CACHE = build_program()
    in_maps = _host_prep(inputs, Wq, Wk, Wv)
    # The first execution after a fresh compile occasionally hits a
    # transient NRT_EXEC_UNIT_UNRECOVERABLE; a retry reliably succeeds.
    last_err = None
    Z = None
    for _ in range(3):
        try:
            res = run_bass_kernel_spmd(
                _NC_CACHE, in_maps, list(range(N_CORES)))
            Z = _host_gather(res.results)
            if np.isfinite(Z).all():
                return Z
        except Exception as e:  # noqa: BLE001
            last_err = e
    if Z is not None:
        return Z
    raise last_err


# revision 30
# speedup vs baseline: 1.0087x; 1.0087x over previous
"""Causal self-attention (single head) on 8 Trainium2 NeuronCores.

Sharding: 8 cores = 4 batches x 2 query-tile parity sets. Core c handles
batch (c % 4). Cores 0-3 take query tiles t in {15,13,...,1} (128 rows
each), cores 4-7 take t in {14,12,...,0}. Attention iteration i uses a
fixed causal extent E(i) = 16-2i k-tiles, so a single SPMD program
serves all cores; even-parity cores waste one fully-masked k-tile per
iteration.

Math: scores = x Wq^T Wk x^T = (x A) x^T with A = Wq^T Wk folded on the
host. The kernel computes Q'^T = A^T-blocks x x_q^T (64 units over the
core's own 1024 q rows) instead of the key-side G = A x^T (128 units
over all 2048 keys) -- halving the projection phase.

Pair-swap trick: for odd-parity cores the host swaps the two 128-col
tiles inside each 256-col half-quarter of x^T (and permutes the xn
k-tiles to match), so BOTH parities' own q-tiles sit at column offsets
{0, 256} of every 512-col quarter. P1 then reads its moving operand
directly out of the resident key matrix (xk_all) through a strided AP
and the separate 2MB x_q staging load disappears; each 256-col half
still holds an adjacent tile pair, so every causal extent stays
covered, the mask still lands on the last 256 columns of each extent
(odd cores: [tri|0], even cores: [tri|-inf]), and the q-tile just
lands pair-swapped in qT_sb (consumers index i^1).

All transposes (softmax weights W and U = W x) go through the DMA xbar
instead of the PE array. Loads are consolidated into ~17 large
contiguous dma_starts striped across both HWDGE rings: sequencers pay
~0.7us issue per dma_start, completion semaphores lag data by several
us (DMA engine 15 runs ~15% slow and is the long pole), and the first
xbar transpose's guard waits on every previously-emitted DMA, gating
U(0). P1 accumulates dt-outer so each PSUM bank's copy overlaps the
next bank's matmuls. Iterations are software-pipelined: step j emits
scores(j), U(j-1), Z(j-2). Softmax skips max-subtraction (scores/32
stay in a safe exp range); row sums come free via activation
accum_out. Z emits its column halves sequentially so scale+store of
the first overlaps matmuls of the second; the final step splits
quarters to shrink the exposed tail further.
"""

import sys

for _p in ("/opt/trn_rl_repo", "/root/.axon_site/_ro/trn_rl_repo"):
    if _p not in sys.path:
        sys.path.append(_p)

import numpy as np

import concourse.bass as bass  # noqa: F401
import concourse.mybir as mybir
import concourse.tile as tile
from concourse import bacc
from concourse.bass_utils import run_bass_kernel_spmd

F32 = mybir.dt.float32
F16 = mybir.dt.float16

BATCH, SEQ, D, P = 4, 2048, 1024, 1024
N_CORES = 8
QT = 128          # query tile rows
KTL = 128         # key tile
NBLK = 512        # matmul moving free dim
ND = D // 128     # 8 d-tiles
NKT = SEQ // KTL  # 16 k-tiles
NQT = 8           # q-tiles per core
SCALE = 1.0 / float(np.sqrt(P))
NEG = -1e9

# processing order over program q-tile index i (extent = 16-2i). The
# first two steps are the smallest extents so their exp/transpose
# chains clear early; scores stay ~4 steps ahead of U.
ORDER = (7, 6, 0, 1, 2, 3, 4, 5)


def _extent(i):
    return 16 - 2 * i


def _chunks(width):
    out = []
    w = width
    while w >= NBLK:
        out.append(NBLK)
        w -= NBLK
    if w:
        assert w == 256, w
        out.append(256)
    return out


def build_program():
    nc = bacc.Bacc("TRN2", target_bir_lowering=False)

    # All inputs are host-pre-tiled to match the SBUF destination layout
    # exactly, so every DMA load is contiguous on both sides (2-8KB runs
    # per partition, few descriptors). 1KB-run slicing costs ~10us of
    # HWDGE descriptor generation per 2MB and stalls the issuing ring.
    xkq = nc.dram_tensor("xkq", [4, 128, ND, 512], F16, kind="ExternalInput")
    xn2 = nc.dram_tensor("xn2", [128, NKT, D], F16, kind="ExternalInput")
    AT = nc.dram_tensor("AT", [D, D], F16, kind="ExternalInput")
    wv2 = nc.dram_tensor("wv2", [128, ND, P], F16, kind="ExternalInput")
    mask = nc.dram_tensor("mask", [QT, 256], F32, kind="ExternalInput")
    out = nc.dram_tensor("out", [NQT * QT, P], F32, kind="ExternalOutput")

    at_r = AT.rearrange("(et ep) d -> ep et d", ep=128)

    with tile.TileContext(nc) as tc:
        with (
            tc.tile_pool(name="resident", bufs=1) as resident,
            tc.tile_pool(name="wrow", bufs=2) as wrow,
            tc.tile_pool(name="tpool", bufs=2) as tpool,
            tc.tile_pool(name="small", bufs=6) as small,
            tc.tile_pool(name="outp", bufs=2) as outp,
        ):
            at_sb = resident.tile([128, ND, D], F16)      # A' = Wq^T Wk
            qT_sb = resident.tile([128, ND, NQT * QT], F16)   # Q'^T [d, q]
            xk_all = resident.tile([128, 4, ND, 512], F16)  # x.T, k-quarters
            xn_all = resident.tile([128, NKT, D], F16)    # x natural [k, d]
            wv_sb = resident.tile([128, ND, P], F16)
            mask_sb = resident.tile([QT, 256], F32)
            cbias = resident.tile([QT, 1], F32)
            nc.vector.memset(cbias, -4.0)

            # Consolidated loads, striped across both rings in consumer
            # order. P1 runs et-outer inside each xk quarter, consuming
            # (at[et], xk-quarter pieces) as they stream: q0 + all at
            # slices lead (P1's every quarter touches every at[et]),
            # then q1-q3 split in halves so their completion semaphores
            # land ahead of P1's ~8.5us-per-quarter pace; xn/wv follow
            # for U(7)/Z(7) at ~50us.
            nc.sync.dma_start(out=xk_all[:, 0, 0:2, :], in_=xkq[0, :, 0:2, :])
            nc.scalar.dma_start(out=xk_all[:, 0, 2:4, :],
                                in_=xkq[0, :, 2:4, :])
            nc.sync.dma_start(out=at_sb[:, 0, 0:256], in_=at_r[:, 0, 0:256])
            nc.sync.dma_start(out=at_sb[:, 0, 256:D], in_=at_r[:, 0, 256:D])
            nc.scalar.dma_start(out=at_sb[:, 1, :], in_=at_r[:, 1, :])
            nc.sync.dma_start(out=at_sb[:, 2, :], in_=at_r[:, 2, :])
            nc.scalar.dma_start(out=xk_all[:, 0, 4:ND, :],
                                in_=xkq[0, :, 4:ND, :])
            nc.sync.dma_start(out=at_sb[:, 4, :], in_=at_r[:, 4, :])
            nc.scalar.dma_start(out=at_sb[:, 3, :], in_=at_r[:, 3, :])
            nc.sync.dma_start(out=at_sb[:, 6, :], in_=at_r[:, 6, :])
            nc.scalar.dma_start(out=at_sb[:, 5, :], in_=at_r[:, 5, :])
            nc.scalar.dma_start(out=at_sb[:, 7, :], in_=at_r[:, 7, :])
            nc.sync.dma_start(out=xk_all[:, 1, 0:4, :], in_=xkq[1, :, 0:4, :])
            nc.sync.dma_start(out=xk_all[:, 1, 4:ND, :],
                              in_=xkq[1, :, 4:ND, :])
            nc.scalar.dma_start(out=mask_sb, in_=mask[:, :])
            nc.scalar.dma_start(out=xk_all[:, 2, 0:4, :],
                                in_=xkq[2, :, 0:4, :])
            nc.scalar.dma_start(out=xk_all[:, 2, 4:ND, :],
                                in_=xkq[2, :, 4:ND, :])
            nc.sync.dma_start(out=xk_all[:, 3, 0:4, :], in_=xkq[3, :, 0:4, :])
            nc.sync.dma_start(out=xk_all[:, 3, 4:ND, :],
                              in_=xkq[3, :, 4:ND, :])
            nc.scalar.dma_start(out=xn_all[:, 0:8, :], in_=xn2[:, 0:8, :])
            nc.sync.dma_start(out=xn_all[:, 8:NKT, :], in_=xn2[:, 8:NKT, :])
            nc.sync.dma_start(out=wv_sb[:, 0:4, :], in_=wv2[:, 0:4, :])
            nc.scalar.dma_start(out=wv_sb[:, 4:ND, :], in_=wv2[:, 4:ND, :])

            # --- Q'^T = A'-blocks x x_q, with x_q read straight out of
            # xk_all: quarter qq holds the core's own q-tiles at column
            # offsets 0 (program i=7-2qq) and 256 (i=6-2qq). et-outer
            # inside each quarter so the matmuls chase the streaming
            # at[et]/xk pieces; the 8 dt-banks complete together at the
            # quarter's end and their single [128,256] copies overlap
            # the next quarter's matmuls. The copy lands the pair
            # contiguously at qT columns (6-2qq)*128 (consumers index
            # i^1).
            # Quarters are processed in pairs (2j, 2j+1): for each (et,
            # dt) the stationary at-block loads ONCE and serves two
            # back-to-back N=256 matmuls (one per quarter), so
            # LDWEIGHTS fully hides and the pair streams at the
            # 256-cycle floor.
            with tc.tile_pool(name="p1ps", bufs=8, space="PSUM") as p1ps:
                for qq in range(4):
                    xq_ap = xk_all[:, qq, :, :].rearrange(
                        "ep et (h c) -> ep et h c", c=128)
                    banks = [p1ps.tile([128, 256], F32, tag="p1",
                                       name=f"p1b{qq}_{dt}")
                             for dt in range(ND)]
                    for et in range(ND):
                        for dt in range(ND):
                            nc.tensor.matmul(
                                banks[dt],
                                at_sb[:, et, dt * 128:(dt + 1) * 128],
                                xq_ap[:, et, 0:3:2, :],
                                start=(et == 0),
                                stop=(et == ND - 1),
                            )
                    c0 = (6 - 2 * qq) * QT
                    for dt in range(ND):
                        cp = (nc.scalar.copy if dt % 2 == 0
                              else nc.vector.tensor_copy)
                        cp(qT_sb[:, dt, c0:c0 + 256], banks[dt])

            # --- attention, software-pipelined: scores(j), U(j-1), Z(j-2)
            with (
                tc.tile_pool(name="sps", bufs=4, space="PSUM") as sps,
                tc.tile_pool(name="uzps", bufs=4, space="PSUM") as uzps,
            ):
                state = {}

                def emit_scores(i):
                    ext = _extent(i)
                    width = ext * KTL
                    iq = i ^ 1  # qT_sb stores each quarter's pair swapped
                    s_list = []
                    off = 0
                    for cw in _chunks(width):
                        s_ps = sps.tile([QT, NBLK], F32, tag="s")
                        psv = s_ps[:, :cw]
                        for dt in range(ND):
                            nc.tensor.matmul(
                                psv,
                                qT_sb[:, dt, iq * QT:(iq + 1) * QT],
                                xk_all[:, off // 512, dt, 0:cw],
                                start=(dt == 0),
                                stop=(dt == ND - 1),
                            )
                        s_list.append((psv, off, cw))
                        off += cw

                    # additive causal mask on the last 256 columns
                    last_ps, _, cw_l = s_list[-1]
                    nc.vector.tensor_add(
                        last_ps[:, cw_l - 256:cw_l],
                        last_ps[:, cw_l - 256:cw_l],
                        mask_sb,
                    )

                    # exp((s + m) * scale - 4) -> fp16 weights row; row
                    # sums free via accum_out
                    w_sb = wrow.tile([QT, SEQ], F16, tag="w")
                    lparts = small.tile([QT, 4], F32, tag="lp")
                    for ci, (psv, off_c, cw) in enumerate(s_list):
                        nc.scalar.activation(
                            w_sb[:, off_c:off_c + cw],
                            psv,
                            mybir.ActivationFunctionType.Exp,
                            scale=SCALE,
                            bias=cbias,
                            accum_out=lparts[:, ci:ci + 1],
                        )
                    lsum = small.tile([QT, 1], F32, tag="ls")
                    nc.vector.reduce_sum(
                        lsum, lparts[:, :len(s_list)],
                        axis=mybir.AxisListType.X)
                    rl = small.tile([QT, 1], F32, tag="rl")
                    nc.vector.reciprocal(rl, lsum)

                    # blocked transpose W -> W^T tiles via the DMA xbar
                    wT = tpool.tile([128, NKT, 128], F16, tag="wT")
                    nc.sync.dma_start_transpose(
                        out=wT[:, 0:ext, :], in_=w_sb[:, 0:width])
                    state[i] = dict(ext=ext, wT=wT, rl=rl)

                def emit_U(i):
                    st = state[i]
                    ext = st["ext"]
                    wT = st["wT"]
                    u0 = uzps.tile([QT, NBLK], F32, tag="uz")
                    u1 = uzps.tile([QT, NBLK], F32, tag="uz")
                    for kt in range(ext):
                        nc.tensor.matmul(
                            u0, wT[:, kt, :], xn_all[:, kt, 0:NBLK],
                            start=(kt == 0), stop=(kt == ext - 1),
                        )
                        nc.tensor.matmul(
                            u1, wT[:, kt, :], xn_all[:, kt, NBLK:D],
                            start=(kt == 0), stop=(kt == ext - 1),
                        )
                    u_sb = wrow.tile([QT, D], F16, tag="u")
                    nc.scalar.copy(u_sb[:, 0:NBLK], u0)
                    nc.vector.tensor_copy(u_sb[:, NBLK:D], u1)
                    uT = tpool.tile([128, ND, 128], F16, tag="uT")
                    nc.sync.dma_start_transpose(out=uT, in_=u_sb)
                    st["uT"] = uT

                def emit_Z(i, last=False):
                    st = state.pop(i)
                    uT = st["uT"]
                    rl = st["rl"]
                    o_sb = outp.tile([QT, P], F32, tag="o")
                    # column-chunked: each chunk's matmuls complete, then
                    # its scale+store overlap the next chunk's matmuls.
                    # The final step uses 4 chunks to shrink the exposed
                    # scale+store tail after the very last matmul.
                    csz = 256 if last else NBLK
                    for cc in range(P // csz):
                        zfull = uzps.tile([QT, NBLK], F32, tag="uz")
                        zc = zfull[:, 0:csz]
                        lo = cc * csz
                        for dt in range(ND):
                            nc.tensor.matmul(
                                zc, uT[:, dt, :],
                                wv_sb[:, dt, lo:lo + csz],
                                start=(dt == 0), stop=(dt == ND - 1),
                            )
                        nc.vector.tensor_scalar_mul(
                            o_sb[:, lo:lo + csz], zc, rl)
                        # scalar queue: keeps output stores from delaying
                        # the dma transposes on the sync queue
                        nc.scalar.dma_start(
                            out=out[i * QT:(i + 1) * QT, lo:lo + csz],
                            in_=o_sb[:, lo:lo + csz])

                for j in range(NQT):
                    emit_scores(ORDER[j])
                    if j >= 1:
                        emit_U(ORDER[j - 1])
                    if j >= 2:
                        emit_Z(ORDER[j - 2])
                emit_U(ORDER[NQT - 1])
                emit_Z(ORDER[NQT - 2])
                emit_Z(ORDER[NQT - 1], last=True)

    nc.compile()
    return nc


def _tiles_for_core(c):
    """Global 128-row query-tile indices, indexed by program i=0..7."""
    return [(15 - 2 * i) if c < 4 else (14 - 2 * i) for i in range(NQT)]


_PSWAP = (1, 0, 3, 2)


def _host_prep(inputs, Wq, Wk, Wv):
    x = np.asarray(inputs, dtype=np.float32)
    Wqf = np.asarray(Wq, dtype=np.float32)
    Wkf = np.asarray(Wk, dtype=np.float32)
    # scores = x A' x^T with A' = Wq^T Wk; the Q'^T matmul contracts over
    # A's rows (lhsT[e, d] = A'[e, d]), so A' itself is the stationary.
    ATm = np.ascontiguousarray((Wqf.T @ Wkf).astype(np.float16))
    WvT = np.ascontiguousarray(
        np.asarray(Wv, dtype=np.float32).T.astype(np.float16))

    qi = np.arange(QT)[:, None]
    ki = np.arange(128)[None, :]
    tri = np.where(qi >= ki, 0.0, NEG).astype(np.float32)
    # pair-swapped (odd-tile) cores: the 256-col tail of every extent is
    # [diag tile | older allowed tile]; natural (even-tile) cores:
    # [diag tile | future masked tile].
    mask_odd = np.concatenate([tri, np.zeros((QT, 128), np.float32)], axis=1)
    mask_evn = np.concatenate(
        [tri, np.full((QT, 128), NEG, np.float32)], axis=1)

    # pre-tile the bulk tensors to match the SBUF destination layouts so
    # the device DMA loads are contiguous on both sides
    wv2 = np.ascontiguousarray(
        WvT.reshape(ND, 128, P).transpose(1, 0, 2))  # [128, dt, p]

    in_maps = []
    cache = {}
    for c in range(N_CORES):
        b = c % 4
        swap = c < 4  # odd global tiles at offsets {128,384} -> pair-swap
        key = (b, swap)
        if key not in cache:
            xTb = x[b].T.astype(np.float16)              # [d, k]
            xq4 = xTb.reshape(ND, 128, 4, 4, 128)        # [et,ep,qq,h,128]
            if swap:
                xq4 = xq4[:, :, :, _PSWAP, :]
            xkq = np.ascontiguousarray(
                xq4.reshape(ND, 128, 4, 512).transpose(2, 1, 0, 3))
            xnat = x[b].astype(np.float16).reshape(NKT, 128, D)
            if swap:
                perm = [4 * (kt // 4) + _PSWAP[kt % 4] for kt in range(NKT)]
                xnat = xnat[perm]
            xn2 = np.ascontiguousarray(xnat.transpose(1, 0, 2))
            cache[key] = (xkq, xn2)
        xkq, xn2 = cache[key]
        in_maps.append({
            "xkq": xkq,
            "xn2": xn2,
            "AT": ATm,
            "wv2": wv2,
            "mask": mask_odd if swap else mask_evn,
        })
    return in_maps


def _host_gather(results):
    Z = np.empty((BATCH, SEQ, P), dtype=np.float32)
    for c in range(N_CORES):
        b = c % 4
        o = results[c]["out"]
        for i, t in enumerate(_tiles_for_core(c)):
            Z[b, t * QT:(t + 1) * QT, :] = o[i * QT:(i + 1) * QT, :]
    return Z


_NC_CACHE = None


def kernel(inputs, Wq, Wk, Wv):
    global _NC_CACHE
    if _NC_CACHE is None:
        _NC_CACHE = build_program()
    in_maps = _host_prep(inputs, Wq, Wk, Wv)
    # The first execution after a fresh compile occasionally hits a
    # transient NRT_EXEC_UNIT_UNRECOVERABLE; a retry reliably succeeds.
    last_err = None
    Z = None
    for _ in range(3):
        try:
            res = run_bass_kernel_spmd(
                _NC_CACHE, in_maps, list(range(N_CORES)))
            Z = _host_gather(res.results)
            if np.isfinite(Z).all():
                return Z
        except Exception as e:  # noqa: BLE001
            last_err = e
    if Z is not None:
        return Z
    raise last_err


# revision 33
# speedup vs baseline: 1.0123x; 1.0036x over previous
"""Causal self-attention (single head) on 8 Trainium2 NeuronCores.

Sharding: 8 cores = 4 batches x 2 query-tile parity sets. Core c handles
batch (c % 4). Cores 0-3 take query tiles t in {15,13,...,1} (128 rows
each), cores 4-7 take t in {14,12,...,0}. Attention iteration i uses a
fixed causal extent E(i) = 16-2i k-tiles, so a single SPMD program
serves all cores; even-parity cores waste one fully-masked k-tile per
iteration.

Math: scores = x Wq^T Wk x^T = (x A) x^T with A = Wq^T Wk folded on the
host. The kernel computes Q'^T = A^T-blocks x x_q^T (64 units over the
core's own 1024 q rows) instead of the key-side G = A x^T (128 units
over all 2048 keys) -- halving the projection phase.

Pair-swap trick: for odd-parity cores the host swaps the two 128-col
tiles inside each 256-col half-quarter of x^T (and permutes the xn
k-tiles to match), so BOTH parities' own q-tiles sit at column offsets
{0, 256} of every 512-col quarter. P1 then reads its moving operand
directly out of the resident key matrix (xk_all) through a strided AP
and the separate 2MB x_q staging load disappears; each 256-col half
still holds an adjacent tile pair, so every causal extent stays
covered, the mask still lands on the last 256 columns of each extent
(odd cores: [tri|0], even cores: [tri|-inf]), and the q-tile just
lands pair-swapped in qT_sb (consumers index i^1).

All transposes (softmax weights W and U = W x) go through the DMA xbar
instead of the PE array. Loads are consolidated into ~17 large
contiguous dma_starts striped across both HWDGE rings: sequencers pay
~0.7us issue per dma_start, completion semaphores lag data by several
us (DMA engine 15 runs ~15% slow and is the long pole), and the first
xbar transpose's guard waits on every previously-emitted DMA, gating
U(0). P1 accumulates dt-outer so each PSUM bank's copy overlaps the
next bank's matmuls. Iterations are software-pipelined: step j emits
scores(j), U(j-1), Z(j-2). Softmax skips max-subtraction (scores/32
stay in a safe exp range); row sums come free via activation
accum_out. Z emits its column halves sequentially so scale+store of
the first overlaps matmuls of the second; the final step splits
quarters to shrink the exposed tail further.
"""

import sys

for _p in ("/opt/trn_rl_repo", "/root/.axon_site/_ro/trn_rl_repo"):
    if _p not in sys.path:
        sys.path.append(_p)

import numpy as np

import concourse.bass as bass  # noqa: F401
import concourse.mybir as mybir
import concourse.tile as tile
from concourse import bacc
from concourse.bass_utils import run_bass_kernel_spmd

F32 = mybir.dt.float32
F16 = mybir.dt.float16

BATCH, SEQ, D, P = 4, 2048, 1024, 1024
N_CORES = 8
QT = 128          # query tile rows
KTL = 128         # key tile
NBLK = 512        # matmul moving free dim
ND = D // 128     # 8 d-tiles
NKT = SEQ // KTL  # 16 k-tiles
NQT = 8           # q-tiles per core
SCALE = 1.0 / float(np.sqrt(P))
NEG = -1e9

# processing order over program q-tile index i (extent = 16-2i). The
# first two steps are the smallest extents so their exp/transpose
# chains clear early; scores stay ~4 steps ahead of U.
ORDER = (7, 6, 0, 1, 2, 3, 4, 5)


def _extent(i):
    return 16 - 2 * i


def _chunks(width):
    out = []
    w = width
    while w >= NBLK:
        out.append(NBLK)
        w -= NBLK
    if w:
        assert w == 256, w
        out.append(256)
    return out


def build_program():
    nc = bacc.Bacc("TRN2", target_bir_lowering=False)

    # All inputs are host-pre-tiled to match the SBUF destination layout
    # exactly, so every DMA load is contiguous on both sides (2-8KB runs
    # per partition, few descriptors). 1KB-run slicing costs ~10us of
    # HWDGE descriptor generation per 2MB and stalls the issuing ring.
    xkq = nc.dram_tensor("xkq", [4, 128, ND, 512], F16, kind="ExternalInput")
    xn2 = nc.dram_tensor("xn2", [128, NKT, D], F16, kind="ExternalInput")
    AT = nc.dram_tensor("AT", [D, D], F16, kind="ExternalInput")
    wv2 = nc.dram_tensor("wv2", [128, ND, P], F16, kind="ExternalInput")
    mask = nc.dram_tensor("mask", [QT, 256], F32, kind="ExternalInput")
    out = nc.dram_tensor("out", [NQT * QT, P], F32, kind="ExternalOutput")

    at_r = AT.rearrange("(et ep) d -> ep et d", ep=128)

    with tile.TileContext(nc) as tc:
        with (
            tc.tile_pool(name="resident", bufs=1) as resident,
            tc.tile_pool(name="wrow", bufs=2) as wrow,
            tc.tile_pool(name="tpool", bufs=2) as tpool,
            tc.tile_pool(name="small", bufs=6) as small,
            tc.tile_pool(name="outp", bufs=2) as outp,
        ):
            at_sb = resident.tile([128, ND, D], F16)      # A' = Wq^T Wk
            qT_sb = resident.tile([128, ND, NQT * QT], F16)   # Q'^T [d, q]
            xk_all = resident.tile([128, 4, ND, 512], F16)  # x.T, k-quarters
            xn_all = resident.tile([128, NKT, D], F16)    # x natural [k, d]
            wv_sb = resident.tile([128, ND, P], F16)
            mask_sb = resident.tile([QT, 256], F32)
            cbias = resident.tile([QT, 1], F32)
            nc.vector.memset(cbias, -4.0)

            # Consolidated loads, striped across both rings in consumer
            # order. P1 runs et-outer inside each xk quarter, consuming
            # (at[et], xk-quarter pieces) as they stream: q0 + all at
            # slices lead (P1's every quarter touches every at[et]),
            # then q1-q3 split in halves so their completion semaphores
            # land ahead of P1's ~8.5us-per-quarter pace; xn/wv follow
            # for U(7)/Z(7) at ~50us.
            nc.sync.dma_start(out=xk_all[:, 0, 0:2, :], in_=xkq[0, :, 0:2, :])
            nc.scalar.dma_start(out=xk_all[:, 0, 2:4, :],
                                in_=xkq[0, :, 2:4, :])
            nc.sync.dma_start(out=at_sb[:, 0, 0:256], in_=at_r[:, 0, 0:256])
            nc.sync.dma_start(out=at_sb[:, 0, 256:D], in_=at_r[:, 0, 256:D])
            nc.scalar.dma_start(out=at_sb[:, 1, :], in_=at_r[:, 1, :])
            nc.sync.dma_start(out=at_sb[:, 2, :], in_=at_r[:, 2, :])
            nc.scalar.dma_start(out=xk_all[:, 0, 4:ND, :],
                                in_=xkq[0, :, 4:ND, :])
            # q1's first half lands before P1 finishes q0 (its
            # completion semaphore trails data by ~3us)
            nc.sync.dma_start(out=xk_all[:, 1, 0:4, :], in_=xkq[1, :, 0:4, :])
            nc.scalar.dma_start(out=at_sb[:, 3, :], in_=at_r[:, 3, :])
            nc.sync.dma_start(out=at_sb[:, 4, :], in_=at_r[:, 4, :])
            nc.scalar.dma_start(out=at_sb[:, 5, :], in_=at_r[:, 5, :])
            nc.sync.dma_start(out=at_sb[:, 6, :], in_=at_r[:, 6, :])
            nc.scalar.dma_start(out=at_sb[:, 7, :], in_=at_r[:, 7, :])
            nc.sync.dma_start(out=xk_all[:, 1, 4:ND, :],
                              in_=xkq[1, :, 4:ND, :])
            nc.scalar.dma_start(out=mask_sb, in_=mask[:, :])
            nc.scalar.dma_start(out=xk_all[:, 2, 0:4, :],
                                in_=xkq[2, :, 0:4, :])
            nc.scalar.dma_start(out=xk_all[:, 2, 4:ND, :],
                                in_=xkq[2, :, 4:ND, :])
            nc.sync.dma_start(out=xk_all[:, 3, 0:4, :], in_=xkq[3, :, 0:4, :])
            nc.sync.dma_start(out=xk_all[:, 3, 4:ND, :],
                              in_=xkq[3, :, 4:ND, :])
            nc.scalar.dma_start(out=xn_all[:, 0:8, :], in_=xn2[:, 0:8, :])
            nc.sync.dma_start(out=xn_all[:, 8:NKT, :], in_=xn2[:, 8:NKT, :])
            nc.sync.dma_start(out=wv_sb[:, 0:4, :], in_=wv2[:, 0:4, :])
            nc.scalar.dma_start(out=wv_sb[:, 4:ND, :], in_=wv2[:, 4:ND, :])

            # --- Q'^T = A'-blocks x x_q, with x_q read straight out of
            # xk_all: quarter qq holds the core's own q-tiles at column
            # offsets 0 (program i=7-2qq) and 256 (i=6-2qq). et-outer
            # inside each quarter so the matmuls chase the streaming
            # at[et]/xk pieces; the 8 dt-banks complete together at the
            # quarter's end and their single [128,256] copies overlap
            # the next quarter's matmuls. The copy lands the pair
            # contiguously at qT columns (6-2qq)*128 (consumers index
            # i^1).
            # Quarters are processed in pairs (2j, 2j+1): for each (et,
            # dt) the stationary at-block loads ONCE and serves two
            # back-to-back N=256 matmuls (one per quarter), so
            # LDWEIGHTS fully hides and the pair streams at the
            # 256-cycle floor.
            with tc.tile_pool(name="p1ps", bufs=8, space="PSUM") as p1ps:
                for qq in range(4):
                    xq_ap = xk_all[:, qq, :, :].rearrange(
                        "ep et (h c) -> ep et h c", c=128)
                    banks = [p1ps.tile([128, 256], F32, tag="p1",
                                       name=f"p1b{qq}_{dt}")
                             for dt in range(ND)]
                    for et in range(ND):
                        for dt in range(ND):
                            nc.tensor.matmul(
                                banks[dt],
                                at_sb[:, et, dt * 128:(dt + 1) * 128],
                                xq_ap[:, et, 0:3:2, :],
                                start=(et == 0),
                                stop=(et == ND - 1),
                            )
                    c0 = (6 - 2 * qq) * QT
                    for dt in range(ND):
                        cp = (nc.scalar.copy if dt % 2 == 0
                              else nc.vector.tensor_copy)
                        cp(qT_sb[:, dt, c0:c0 + 256], banks[dt])

            # --- attention, software-pipelined: scores(j), U(j-1), Z(j-2)
            with (
                tc.tile_pool(name="sps", bufs=4, space="PSUM") as sps,
                tc.tile_pool(name="uzps", bufs=4, space="PSUM") as uzps,
            ):
                state = {}

                def emit_scores(i):
                    ext = _extent(i)
                    width = ext * KTL
                    iq = i ^ 1  # qT_sb stores each quarter's pair swapped
                    s_list = []
                    off = 0
                    for cw in _chunks(width):
                        s_ps = sps.tile([QT, NBLK], F32, tag="s")
                        psv = s_ps[:, :cw]
                        for dt in range(ND):
                            nc.tensor.matmul(
                                psv,
                                qT_sb[:, dt, iq * QT:(iq + 1) * QT],
                                xk_all[:, off // 512, dt, 0:cw],
                                start=(dt == 0),
                                stop=(dt == ND - 1),
                            )
                        s_list.append((psv, off, cw))
                        off += cw

                    # additive causal mask on the last 256 columns
                    last_ps, _, cw_l = s_list[-1]
                    nc.vector.tensor_add(
                        last_ps[:, cw_l - 256:cw_l],
                        last_ps[:, cw_l - 256:cw_l],
                        mask_sb,
                    )

                    # exp((s + m) * scale - 4) -> fp16 weights row; row
                    # sums free via accum_out
                    w_sb = wrow.tile([QT, SEQ], F16, tag="w")
                    lparts = small.tile([QT, 4], F32, tag="lp")
                    for ci, (psv, off_c, cw) in enumerate(s_list):
                        nc.scalar.activation(
                            w_sb[:, off_c:off_c + cw],
                            psv,
                            mybir.ActivationFunctionType.Exp,
                            scale=SCALE,
                            bias=cbias,
                            accum_out=lparts[:, ci:ci + 1],
                        )
                    lsum = small.tile([QT, 1], F32, tag="ls")
                    nc.vector.reduce_sum(
                        lsum, lparts[:, :len(s_list)],
                        axis=mybir.AxisListType.X)
                    rl = small.tile([QT, 1], F32, tag="rl")
                    nc.vector.reciprocal(rl, lsum)

                    # blocked transpose W -> W^T tiles via the DMA xbar
                    wT = tpool.tile([128, NKT, 128], F16, tag="wT")
                    nc.sync.dma_start_transpose(
                        out=wT[:, 0:ext, :], in_=w_sb[:, 0:width])
                    state[i] = dict(ext=ext, wT=wT, rl=rl)

                def emit_U(i):
                    st = state[i]
                    ext = st["ext"]
                    wT = st["wT"]
                    u0 = uzps.tile([QT, NBLK], F32, tag="uz")
                    u1 = uzps.tile([QT, NBLK], F32, tag="uz")
                    for kt in range(ext):
                        nc.tensor.matmul(
                            u0, wT[:, kt, :], xn_all[:, kt, 0:NBLK],
                            start=(kt == 0), stop=(kt == ext - 1),
                        )
                        nc.tensor.matmul(
                            u1, wT[:, kt, :], xn_all[:, kt, NBLK:D],
                            start=(kt == 0), stop=(kt == ext - 1),
                        )
                    u_sb = wrow.tile([QT, D], F16, tag="u")
                    nc.scalar.copy(u_sb[:, 0:NBLK], u0)
                    nc.vector.tensor_copy(u_sb[:, NBLK:D], u1)
                    uT = tpool.tile([128, ND, 128], F16, tag="uT")
                    nc.sync.dma_start_transpose(out=uT, in_=u_sb)
                    st["uT"] = uT

                def emit_Z(i, last=False):
                    st = state.pop(i)
                    uT = st["uT"]
                    rl = st["rl"]
                    o_sb = outp.tile([QT, P], F32, tag="o")
                    # column-chunked: each chunk's matmuls complete, then
                    # its scale+store overlap the next chunk's matmuls.
                    # The final step uses 4 chunks to shrink the exposed
                    # scale+store tail after the very last matmul.
                    csz = 256 if last else NBLK
                    for cc in range(P // csz):
                        zfull = uzps.tile([QT, NBLK], F32, tag="uz")
                        zc = zfull[:, 0:csz]
                        lo = cc * csz
                        for dt in range(ND):
                            nc.tensor.matmul(
                                zc, uT[:, dt, :],
                                wv_sb[:, dt, lo:lo + csz],
                                start=(dt == 0), stop=(dt == ND - 1),
                            )
                        nc.vector.tensor_scalar_mul(
                            o_sb[:, lo:lo + csz], zc, rl)
                        # scalar queue: keeps output stores from delaying
                        # the dma transposes on the sync queue
                        nc.scalar.dma_start(
                            out=out[i * QT:(i + 1) * QT, lo:lo + csz],
                            in_=o_sb[:, lo:lo + csz])

                for j in range(NQT):
                    emit_scores(ORDER[j])
                    if j >= 1:
                        emit_U(ORDER[j - 1])
                    if j >= 2:
                        emit_Z(ORDER[j - 2])
                emit_U(ORDER[NQT - 1])
                emit_Z(ORDER[NQT - 2])
                emit_Z(ORDER[NQT - 1], last=True)

    nc.compile()
    return nc


def _tiles_for_core(c):
    """Global 128-row query-tile indices, indexed by program i=0..7."""
    return [(15 - 2 * i) if c < 4 else (14 - 2 * i) for i in range(NQT)]


_PSWAP = (1, 0, 3, 2)


def _host_prep(inputs, Wq, Wk, Wv):
    x = np.asarray(inputs, dtype=np.float32)
    Wqf = np.asarray(Wq, dtype=np.float32)
    Wkf = np.asarray(Wk, dtype=np.float32)
    # scores = x A' x^T with A' = Wq^T Wk; the Q'^T matmul contracts over
    # A's rows (lhsT[e, d] = A'[e, d]), so A' itself is the stationary.
    ATm = np.ascontiguousarray((Wqf.T @ Wkf).astype(np.float16))
    WvT = np.ascontiguousarray(
        np.asarray(Wv, dtype=np.float32).T.astype(np.float16))

    qi = np.arange(QT)[:, None]
    ki = np.arange(128)[None, :]
    tri = np.where(qi >= ki, 0.0, NEG).astype(np.float32)
    # pair-swapped (odd-tile) cores: the 256-col tail of every extent is
    # [diag tile | older allowed tile]; natural (even-tile) cores:
    # [diag tile | future masked tile].
    mask_odd = np.concatenate([tri, np.zeros((QT, 128), np.float32)], axis=1)
    mask_evn = np.concatenate(
        [tri, np.full((QT, 128), NEG, np.float32)], axis=1)

    # pre-tile the bulk tensors to match the SBUF destination layouts so
    # the device DMA loads are contiguous on both sides
    wv2 = np.ascontiguousarray(
        WvT.reshape(ND, 128, P).transpose(1, 0, 2))  # [128, dt, p]

    in_maps = []
    cache = {}
    for c in range(N_CORES):
        b = c % 4
        swap = c < 4  # odd global tiles at offsets {128,384} -> pair-swap
        key = (b, swap)
        if key not in cache:
            xTb = x[b].T.astype(np.float16)              # [d, k]
            xq4 = xTb.reshape(ND, 128, 4, 4, 128)        # [et,ep,qq,h,128]
            if swap:
                xq4 = xq4[:, :, :, _PSWAP, :]
            xkq = np.ascontiguousarray(
                xq4.reshape(ND, 128, 4, 512).transpose(2, 1, 0, 3))
            xnat = x[b].astype(np.float16).reshape(NKT, 128, D)
            if swap:
                perm = [4 * (kt // 4) + _PSWAP[kt % 4] for kt in range(NKT)]
                xnat = xnat[perm]
            xn2 = np.ascontiguousarray(xnat.transpose(1, 0, 2))
            cache[key] = (xkq, xn2)
        xkq, xn2 = cache[key]
        in_maps.append({
            "xkq": xkq,
            "xn2": xn2,
            "AT": ATm,
            "wv2": wv2,
            "mask": mask_odd if swap else mask_evn,
        })
    return in_maps


def _host_gather(results):
    Z = np.empty((BATCH, SEQ, P), dtype=np.float32)
    for c in range(N_CORES):
        b = c % 4
        o = results[c]["out"]
        for i, t in enumerate(_tiles_for_core(c)):
            Z[b, t * QT:(t + 1) * QT, :] = o[i * QT:(i + 1) * QT, :]
    return Z


_NC_CACHE = None


def kernel(inputs, Wq, Wk, Wv):
    global _NC_CACHE
    if _NC_CACHE is None:
        _NC_CACHE = build_program()
    in_maps = _host_prep(inputs, Wq, Wk, Wv)
    # The first execution after a fresh compile occasionally hits a
    # transient NRT_EXEC_UNIT_UNRECOVERABLE; a retry reliably succeeds.
    last_err = None
    Z = None
    for _ in range(3):
        try:
            res = run_bass_kernel_spmd(
                _NC_CACHE, in_maps, list(range(N_CORES)))
            Z = _host_gather(res.results)
            if np.isfinite(Z).all():
                return Z
        except Exception as e:  # noqa: BLE001
            last_err = e
    if Z is not None:
        return Z
    raise last_err


# revision 34
# speedup vs baseline: 1.0126x; 1.0004x over previous
"""Causal self-attention (single head) on 8 Trainium2 NeuronCores.

Sharding: 8 cores = 4 batches x 2 query-tile parity sets. Core c handles
batch (c % 4). Cores 0-3 take query tiles t in {15,13,...,1} (128 rows
each), cores 4-7 take t in {14,12,...,0}. Attention iteration i uses a
fixed causal extent E(i) = 16-2i k-tiles, so a single SPMD program
serves all cores; even-parity cores waste one fully-masked k-tile per
iteration.

Math: scores = x Wq^T Wk x^T = (x A) x^T with A = Wq^T Wk folded on the
host. The kernel computes Q'^T = A^T-blocks x x_q^T (64 units over the
core's own 1024 q rows) instead of the key-side G = A x^T (128 units
over all 2048 keys) -- halving the projection phase.

Pair-swap trick: for odd-parity cores the host swaps the two 128-col
tiles inside each 256-col half-quarter of x^T (and permutes the xn
k-tiles to match), so BOTH parities' own q-tiles sit at column offsets
{0, 256} of every 512-col quarter. P1 then reads its moving operand
directly out of the resident key matrix (xk_all) through a strided AP
and the separate 2MB x_q staging load disappears; each 256-col half
still holds an adjacent tile pair, so every causal extent stays
covered, the mask still lands on the last 256 columns of each extent
(odd cores: [tri|0], even cores: [tri|-inf]), and the q-tile just
lands pair-swapped in qT_sb (consumers index i^1).

All transposes (softmax weights W and U = W x) go through the DMA xbar
instead of the PE array. Loads are consolidated into ~17 large
contiguous dma_starts striped across both HWDGE rings: sequencers pay
~0.7us issue per dma_start, completion semaphores lag data by several
us (DMA engine 15 runs ~15% slow and is the long pole), and the first
xbar transpose's guard waits on every previously-emitted DMA, gating
U(0). P1 accumulates dt-outer so each PSUM bank's copy overlaps the
next bank's matmuls. Iterations are software-pipelined: step j emits
scores(j), U(j-1), Z(j-2). Softmax skips max-subtraction (scores/32
stay in a safe exp range); row sums come free via activation
accum_out. Z emits its column halves sequentially so scale+store of
the first overlaps matmuls of the second; the final step splits
quarters to shrink the exposed tail further.
"""

import sys

for _p in ("/opt/trn_rl_repo", "/root/.axon_site/_ro/trn_rl_repo"):
    if _p not in sys.path:
        sys.path.append(_p)

import numpy as np

import concourse.bass as bass  # noqa: F401
import concourse.mybir as mybir
import concourse.tile as tile
from concourse import bacc
from concourse.bass_utils import run_bass_kernel_spmd

F32 = mybir.dt.float32
F16 = mybir.dt.float16

BATCH, SEQ, D, P = 4, 2048, 1024, 1024
N_CORES = 8
QT = 128          # query tile rows
KTL = 128         # key tile
NBLK = 512        # matmul moving free dim
ND = D // 128     # 8 d-tiles
NKT = SEQ // KTL  # 16 k-tiles
NQT = 8           # q-tiles per core
SCALE = 1.0 / float(np.sqrt(P))
NEG = -1e9

# processing order over program q-tile index i (extent = 16-2i). The
# first two steps are the smallest extents so their exp/transpose
# chains clear early; scores stay ~4 steps ahead of U.
ORDER = (7, 6, 0, 1, 2, 3, 4, 5)


def _extent(i):
    return 16 - 2 * i


def _chunks(width):
    out = []
    w = width
    while w >= NBLK:
        out.append(NBLK)
        w -= NBLK
    if w:
        assert w == 256, w
        out.append(256)
    return out


def build_program():
    nc = bacc.Bacc("TRN2", target_bir_lowering=False)

    # All inputs are host-pre-tiled to match the SBUF destination layout
    # exactly, so every DMA load is contiguous on both sides (2-8KB runs
    # per partition, few descriptors). 1KB-run slicing costs ~10us of
    # HWDGE descriptor generation per 2MB and stalls the issuing ring.
    xkq = nc.dram_tensor("xkq", [4, 128, ND, 512], F16, kind="ExternalInput")
    xn2 = nc.dram_tensor("xn2", [128, NKT, D], F16, kind="ExternalInput")
    AT = nc.dram_tensor("AT", [D, D], F16, kind="ExternalInput")
    wv2 = nc.dram_tensor("wv2", [128, ND, P], F16, kind="ExternalInput")
    mask = nc.dram_tensor("mask", [QT, 256], F32, kind="ExternalInput")
    out = nc.dram_tensor("out", [NQT * QT, P], F32, kind="ExternalOutput")

    at_r = AT.rearrange("(et ep) d -> ep et d", ep=128)

    with tile.TileContext(nc) as tc:
        with (
            tc.tile_pool(name="resident", bufs=1) as resident,
            tc.tile_pool(name="wrow", bufs=2) as wrow,
            tc.tile_pool(name="tpool", bufs=2) as tpool,
            tc.tile_pool(name="small", bufs=6) as small,
            tc.tile_pool(name="outp", bufs=2) as outp,
        ):
            at_sb = resident.tile([128, ND, D], F16)      # A' = Wq^T Wk
            qT_sb = resident.tile([128, ND, NQT * QT], F16)   # Q'^T [d, q]
            xk_all = resident.tile([128, 4, ND, 512], F16)  # x.T, k-quarters
            xn_all = resident.tile([128, NKT, D], F16)    # x natural [k, d]
            wv_sb = resident.tile([128, ND, P], F16)
            mask_sb = resident.tile([QT, 256], F32)
            cbias = resident.tile([QT, 1], F32)
            nc.vector.memset(cbias, -4.0)

            # Consolidated loads, striped across both rings in consumer
            # order. P1 runs et-outer inside each xk quarter, consuming
            # (at[et], xk-quarter pieces) as they stream: q0 + all at
            # slices lead (P1's every quarter touches every at[et]),
            # then q1-q3 split in halves so their completion semaphores
            # land ahead of P1's ~8.5us-per-quarter pace; xn/wv follow
            # for U(7)/Z(7) at ~50us.
            nc.sync.dma_start(out=xk_all[:, 0, 0:2, :], in_=xkq[0, :, 0:2, :])
            nc.scalar.dma_start(out=xk_all[:, 0, 2:4, :],
                                in_=xkq[0, :, 2:4, :])
            nc.sync.dma_start(out=at_sb[:, 0, 0:256], in_=at_r[:, 0, 0:256])
            nc.sync.dma_start(out=at_sb[:, 0, 256:D], in_=at_r[:, 0, 256:D])
            nc.scalar.dma_start(out=at_sb[:, 1, :], in_=at_r[:, 1, :])
            nc.sync.dma_start(out=at_sb[:, 2, :], in_=at_r[:, 2, :])
            nc.scalar.dma_start(out=xk_all[:, 0, 4:ND, :],
                                in_=xkq[0, :, 4:ND, :])
            # q1's first half lands before P1 finishes q0 (its
            # completion semaphore trails data by ~3us)
            nc.sync.dma_start(out=xk_all[:, 1, 0:4, :], in_=xkq[1, :, 0:4, :])
            nc.scalar.dma_start(out=at_sb[:, 3, :], in_=at_r[:, 3, :])
            nc.sync.dma_start(out=at_sb[:, 4, :], in_=at_r[:, 4, :])
            nc.scalar.dma_start(out=at_sb[:, 5, :], in_=at_r[:, 5, :])
            nc.sync.dma_start(out=at_sb[:, 6, :], in_=at_r[:, 6, :])
            nc.scalar.dma_start(out=at_sb[:, 7, :], in_=at_r[:, 7, :])
            nc.sync.dma_start(out=xk_all[:, 1, 4:ND, :],
                              in_=xkq[1, :, 4:ND, :])
            nc.scalar.dma_start(out=mask_sb, in_=mask[:, :])
            nc.scalar.dma_start(out=xk_all[:, 2, 0:4, :],
                                in_=xkq[2, :, 0:4, :])
            nc.scalar.dma_start(out=xk_all[:, 2, 4:ND, :],
                                in_=xkq[2, :, 4:ND, :])
            nc.sync.dma_start(out=xk_all[:, 3, 0:4, :], in_=xkq[3, :, 0:4, :])
            nc.sync.dma_start(out=xk_all[:, 3, 4:ND, :],
                              in_=xkq[3, :, 4:ND, :])
            nc.scalar.dma_start(out=xn_all[:, 0:8, :], in_=xn2[:, 0:8, :])
            nc.sync.dma_start(out=xn_all[:, 8:NKT, :], in_=xn2[:, 8:NKT, :])
            nc.sync.dma_start(out=wv_sb[:, 0:4, :], in_=wv2[:, 0:4, :])
            nc.scalar.dma_start(out=wv_sb[:, 4:ND, :], in_=wv2[:, 4:ND, :])

            # --- Q'^T = A'-blocks x x_q, with x_q read straight out of
            # xk_all: quarter qq holds the core's own q-tiles at column
            # offsets 0 (program i=7-2qq) and 256 (i=6-2qq). et-outer
            # inside each quarter so the matmuls chase the streaming
            # at[et]/xk pieces; the 8 dt-banks complete together at the
            # quarter's end and their single [128,256] copies overlap
            # the next quarter's matmuls. The copy lands the pair
            # contiguously at qT columns (6-2qq)*128 (consumers index
            # i^1).
            # Quarters are processed in pairs (2j, 2j+1): for each (et,
            # dt) the stationary at-block loads ONCE and serves two
            # back-to-back N=256 matmuls (one per quarter), so
            # LDWEIGHTS fully hides and the pair streams at the
            # 256-cycle floor.
            with tc.tile_pool(name="p1ps", bufs=8, space="PSUM") as p1ps:
                for qq in range(4):
                    xq_ap = xk_all[:, qq, :, :].rearrange(
                        "ep et (h c) -> ep et h c", c=128)
                    banks = [p1ps.tile([128, 256], F32, tag="p1",
                                       name=f"p1b{qq}_{dt}")
                             for dt in range(ND)]
                    for et in range(ND):
                        for dt in range(ND):
                            nc.tensor.matmul(
                                banks[dt],
                                at_sb[:, et, dt * 128:(dt + 1) * 128],
                                xq_ap[:, et, 0:3:2, :],
                                start=(et == 0),
                                stop=(et == ND - 1),
                            )
                    c0 = (6 - 2 * qq) * QT
                    for dt in range(ND):
                        # last quarter's copies all on ACT so the DVE is
                        # free to run scores(7)'s mask-add immediately
                        # (that add gates exp -> transpose -> U(7))
                        cp = (nc.scalar.copy if (qq == 3 or dt % 2 == 0)
                              else nc.vector.tensor_copy)
                        cp(qT_sb[:, dt, c0:c0 + 256], banks[dt])

            # --- attention, software-pipelined: scores(j), U(j-1), Z(j-2)
            with (
                tc.tile_pool(name="sps", bufs=4, space="PSUM") as sps,
                tc.tile_pool(name="uzps", bufs=4, space="PSUM") as uzps,
            ):
                state = {}

                def emit_scores(i):
                    ext = _extent(i)
                    width = ext * KTL
                    iq = i ^ 1  # qT_sb stores each quarter's pair swapped
                    s_list = []
                    off = 0
                    for cw in _chunks(width):
                        s_ps = sps.tile([QT, NBLK], F32, tag="s")
                        psv = s_ps[:, :cw]
                        for dt in range(ND):
                            nc.tensor.matmul(
                                psv,
                                qT_sb[:, dt, iq * QT:(iq + 1) * QT],
                                xk_all[:, off // 512, dt, 0:cw],
                                start=(dt == 0),
                                stop=(dt == ND - 1),
                            )
                        s_list.append((psv, off, cw))
                        off += cw

                    # additive causal mask on the last 256 columns
                    last_ps, _, cw_l = s_list[-1]
                    nc.vector.tensor_add(
                        last_ps[:, cw_l - 256:cw_l],
                        last_ps[:, cw_l - 256:cw_l],
                        mask_sb,
                    )

                    # exp((s + m) * scale - 4) -> fp16 weights row; row
                    # sums free via accum_out
                    w_sb = wrow.tile([QT, SEQ], F16, tag="w")
                    lparts = small.tile([QT, 4], F32, tag="lp")
                    for ci, (psv, off_c, cw) in enumerate(s_list):
                        nc.scalar.activation(
                            w_sb[:, off_c:off_c + cw],
                            psv,
                            mybir.ActivationFunctionType.Exp,
                            scale=SCALE,
                            bias=cbias,
                            accum_out=lparts[:, ci:ci + 1],
                        )
                    lsum = small.tile([QT, 1], F32, tag="ls")
                    nc.vector.reduce_sum(
                        lsum, lparts[:, :len(s_list)],
                        axis=mybir.AxisListType.X)
                    rl = small.tile([QT, 1], F32, tag="rl")
                    nc.vector.reciprocal(rl, lsum)

                    # blocked transpose W -> W^T tiles via the DMA xbar
                    wT = tpool.tile([128, NKT, 128], F16, tag="wT")
                    nc.sync.dma_start_transpose(
                        out=wT[:, 0:ext, :], in_=w_sb[:, 0:width])
                    state[i] = dict(ext=ext, wT=wT, rl=rl)

                def emit_U(i):
                    st = state[i]
                    ext = st["ext"]
                    wT = st["wT"]
                    u0 = uzps.tile([QT, NBLK], F32, tag="uz")
                    u1 = uzps.tile([QT, NBLK], F32, tag="uz")
                    for kt in range(ext):
                        nc.tensor.matmul(
                            u0, wT[:, kt, :], xn_all[:, kt, 0:NBLK],
                            start=(kt == 0), stop=(kt == ext - 1),
                        )
                        nc.tensor.matmul(
                            u1, wT[:, kt, :], xn_all[:, kt, NBLK:D],
                            start=(kt == 0), stop=(kt == ext - 1),
                        )
                    u_sb = wrow.tile([QT, D], F16, tag="u")
                    nc.scalar.copy(u_sb[:, 0:NBLK], u0)
                    nc.vector.tensor_copy(u_sb[:, NBLK:D], u1)
                    uT = tpool.tile([128, ND, 128], F16, tag="uT")
                    nc.sync.dma_start_transpose(out=uT, in_=u_sb)
                    st["uT"] = uT

                def emit_Z(i, last=False):
                    st = state.pop(i)
                    uT = st["uT"]
                    rl = st["rl"]
                    o_sb = outp.tile([QT, P], F32, tag="o")
                    # column-chunked: each chunk's matmuls complete, then
                    # its scale+store overlap the next chunk's matmuls.
                    # The final step uses 4 chunks to shrink the exposed
                    # scale+store tail after the very last matmul.
                    csz = 256 if last else NBLK
                    for cc in range(P // csz):
                        zfull = uzps.tile([QT, NBLK], F32, tag="uz")
                        zc = zfull[:, 0:csz]
                        lo = cc * csz
                        for dt in range(ND):
                            nc.tensor.matmul(
                                zc, uT[:, dt, :],
                                wv_sb[:, dt, lo:lo + csz],
                                start=(dt == 0), stop=(dt == ND - 1),
                            )
                        nc.vector.tensor_scalar_mul(
                            o_sb[:, lo:lo + csz], zc, rl)
                        # scalar queue: keeps output stores from delaying
                        # the dma transposes on the sync queue
                        nc.scalar.dma_start(
                            out=out[i * QT:(i + 1) * QT, lo:lo + csz],
                            in_=o_sb[:, lo:lo + csz])

                for j in range(NQT):
                    emit_scores(ORDER[j])
                    if j >= 1:
                        emit_U(ORDER[j - 1])
                    if j >= 2:
                        emit_Z(ORDER[j - 2])
                emit_U(ORDER[NQT - 1])
                emit_Z(ORDER[NQT - 2])
                emit_Z(ORDER[NQT - 1], last=True)

    nc.compile()
    return nc


def _tiles_for_core(c):
    """Global 128-row query-tile indices, indexed by program i=0..7."""
    return [(15 - 2 * i) if c < 4 else (14 - 2 * i) for i in range(NQT)]


_PSWAP = (1, 0, 3, 2)


def _host_prep(inputs, Wq, Wk, Wv):
    x = np.asarray(inputs, dtype=np.float32)
    Wqf = np.asarray(Wq, dtype=np.float32)
    Wkf = np.asarray(Wk, dtype=np.float32)
    # scores = x A' x^T with A' = Wq^T Wk; the Q'^T matmul contracts over
    # A's rows (lhsT[e, d] = A'[e, d]), so A' itself is the stationary.
    ATm = np.ascontiguousarray((Wqf.T @ Wkf).astype(np.float16))
    WvT = np.ascontiguousarray(
        np.asarray(Wv, dtype=np.float32).T.astype(np.float16))

    qi = np.arange(QT)[:, None]
    ki = np.arange(128)[None, :]
    tri = np.where(qi >= ki, 0.0, NEG).astype(np.float32)
    # pair-swapped (odd-tile) cores: the 256-col tail of every extent is
    # [diag tile | older allowed tile]; natural (even-tile) cores:
    # [diag tile | future masked tile].
    mask_odd = np.concatenate([tri, np.zeros((QT, 128), np.float32)], axis=1)
    mask_evn = np.concatenate(
        [tri, np.full((QT, 128), NEG, np.float32)], axis=1)

    # pre-tile the bulk tensors to match the SBUF destination layouts so
    # the device DMA loads are contiguous on both sides
    wv2 = np.ascontiguousarray(
        WvT.reshape(ND, 128, P).transpose(1, 0, 2))  # [128, dt, p]

    in_maps = []
    cache = {}
    for c in range(N_CORES):
        b = c % 4
        swap = c < 4  # odd global tiles at offsets {128,384} -> pair-swap
        key = (b, swap)
        if key not in cache:
            xTb = x[b].T.astype(np.float16)              # [d, k]
            xq4 = xTb.reshape(ND, 128, 4, 4, 128)        # [et,ep,qq,h,128]
            if swap:
                xq4 = xq4[:, :, :, _PSWAP, :]
            xkq = np.ascontiguousarray(
                xq4.reshape(ND, 128, 4, 512).transpose(2, 1, 0, 3))
            xnat = x[b].astype(np.float16).reshape(NKT, 128, D)
            if swap:
                perm = [4 * (kt // 4) + _PSWAP[kt % 4] for kt in range(NKT)]
                xnat = xnat[perm]
            xn2 = np.ascontiguousarray(xnat.transpose(1, 0, 2))
            cache[key] = (xkq, xn2)
        xkq, xn2 = cache[key]
        in_maps.append({
            "xkq": xkq,
            "xn2": xn2,
            "AT": ATm,
            "wv2": wv2,
            "mask": mask_odd if swap else mask_evn,
        })
    return in_maps


def _host_gather(results):
    Z = np.empty((BATCH, SEQ, P), dtype=np.float32)
    for c in range(N_CORES):
        b = c % 4
        o = results[c]["out"]
        for i, t in enumerate(_tiles_for_core(c)):
            Z[b, t * QT:(t + 1) * QT, :] = o[i * QT:(i + 1) * QT, :]
    return Z


_NC_CACHE = None


def kernel(inputs, Wq, Wk, Wv):
    global _NC_CACHE
    if _NC_CACHE is None:
        _NC_CACHE = build_program()
    in_maps = _host_prep(inputs, Wq, Wk, Wv)
    # The first execution after a fresh compile occasionally hits a
    # transient NRT_EXEC_UNIT_UNRECOVERABLE; a retry reliably succeeds.
    last_err = None
    Z = None
    for _ in range(3):
        try:
            res = run_bass_kernel_spmd(
                _NC_CACHE, in_maps, list(range(N_CORES)))
            Z = _host_gather(res.results)
            if np.isfinite(Z).all():
                return Z
        except Exception as e:  # noqa: BLE001
            last_err = e
    if Z is not None:
        return Z
    raise last_err
